# revision 1
# baseline (speedup 1.0000x reference)
"""3-layer GraphSAGE (PyG SAGEConv, mean aggregation) on 8 Trainium2 NeuronCores.

Strategy (edge-cut graph partition, per the sharding hint):
  - Destination nodes sharded contiguously across 8 cores (12500 rows each).
  - Mean aggregation is linear, so each layer first projects its own node
    rows through Wl (x @ Wl), all-gathers the projected table (the halo
    exchange), then gathers per-edge source rows with dma_gather and
    segment-sums them on the tensor engine using one-hot selection-matrix
    matmuls accumulated in PSUM.
  - Degrees are computed on-device by an extra matmul against a ones vector
    during layer 0 (reused for all layers).
  - Edges are grouped by (dst 128-tile, source 25000-row chunk) so gather
    indices fit int16; groups are padded to a cross-core-uniform size so a
    single SPMD program serves all 8 cores.

kernel(**inputs) takes the full unsharded inputs and returns the full
(x_final, out, g) tuple, matching the reference.
"""

import numpy as np
import ml_dtypes

import concourse.bacc as bacc
import concourse.tile as tile
import concourse.mybir as mybir
from concourse import library_config
from concourse.bass_utils import run_bass_kernel_spmd

P = 128

CFG = dict(
    n_nodes=100000,
    in_c=256,
    hid_c=128,
    out_c=64,
    n_cores=8,
    chunk=25000,
)


def _cdiv(a, b):
    return -(-a // b)


def _preprocess(edge_index, cfg):
    """Host-side integer graph preprocessing: partition edges by dst owner,
    group by (dst tile, src chunk), sort by src, pad to uniform sizes.
    Returns per-core int16 gather indices (wrapped layout) and bf16 local-dst
    arrays, plus the (shared) padded group size table."""
    n_nodes = cfg["n_nodes"]
    n_cores = cfg["n_cores"]
    chunk = cfg["chunk"]
    npc = n_nodes // n_cores
    n_tiles = _cdiv(npc, P)
    n_chunks = _cdiv(n_nodes, chunk)
    n_groups = n_tiles * n_chunks

    src = edge_index[0].astype(np.int64)
    dst = edge_index[1].astype(np.int64)
    owner = dst // npc

    per_core = []
    G = np.zeros((n_cores, n_groups), np.int64)
    for k in range(n_cores):
        m = owner == k
        s = src[m]
        d = dst[m] - k * npc
        t = d // P
        c = s // chunk
        g = t * n_chunks + c
        o = np.lexsort((s, g))
        s, d, g = s[o], d[o], g[o]
        cnt = np.bincount(g, minlength=n_groups)
        G[k] = cnt
        per_core.append((s, d, g, cnt))

    # padded group sizes, uniform across cores, multiples of 128
    S = (_cdiv_arr(G.max(axis=0), P) * P).astype(np.int64)
    off = np.zeros_like(S)
    off[1:] = np.cumsum(S)[:-1]
    L = int(S.sum())

    idx_list, dloc_list = [], []
    for k in range(n_cores):
        s, d, g, cnt = per_core[k]
        gstart = np.zeros(n_groups, np.int64)
        gstart[1:] = np.cumsum(cnt)[:-1]
        pos = off[g] + (np.arange(len(s)) - gstart[g])
        lsrc = (s - (g % n_chunks) * chunk).astype(np.int16)
        idxw = np.zeros((16, L // 16), np.int16)
        idxw[pos % 16, pos // 16] = lsrc
        idx_list.append(np.tile(idxw, (8, 1)))
        dl = np.full((P, L // P), 255.0, np.float32)
        dl[pos % P, pos // P] = (d % P).astype(np.float32)
        dloc_list.append(dl.astype(ml_dtypes.bfloat16))

    return S, off, L, idx_list, dloc_list


def _cdiv_arr(a, b):
    return -(-a // b)


def _build(S, off, L, cfg):
    """Build the SPMD Bass program (identical for all 8 cores)."""
    dt = mybir.dt
    n_nodes = cfg["n_nodes"]
    n_cores = cfg["n_cores"]
    chunk = cfg["chunk"]
    in_c = cfg["in_c"]
    hid_c = cfg["hid_c"]
    out_c = cfg["out_c"]
    npc = n_nodes // n_cores
    n_tiles = _cdiv(npc, P)
    rows_last = npc - (n_tiles - 1) * P
    n_chunks = _cdiv(n_nodes, chunk)
    Lb = L // P
    nbmax = int((S // P).max())

    nc = bacc.Bacc("TRN2", target_bir_lowering=False)

    x_own = nc.dram_tensor("x_own", [npc, in_c], dt.float32, kind="ExternalInput")
    idx_in = nc.dram_tensor("idx", [128, L // 16], dt.int16, kind="ExternalInput")
    dloc_in = nc.dram_tensor("dloc", [128, Lb], dt.bfloat16, kind="ExternalInput")
    iota_bf_in = nc.dram_tensor("iotabf", [128, nbmax * 128], dt.bfloat16, kind="ExternalInput")
    iota_f_in = nc.dram_tensor("iotaf", [128, nbmax * 128], dt.float32, kind="ExternalInput")
    ident_in = nc.dram_tensor("ident", [128, 128], dt.float32, kind="ExternalInput")
    wl0_in = nc.dram_tensor("Wl0", [in_c, hid_c], dt.float32, kind="ExternalInput")
    wr0_in = nc.dram_tensor("Wr0", [in_c, hid_c], dt.float32, kind="ExternalInput")
    b0_in = nc.dram_tensor("b0", [1, hid_c], dt.float32, kind="ExternalInput")
    wl1_in = nc.dram_tensor("Wl1", [hid_c, hid_c], dt.float32, kind="ExternalInput")
    wr1_in = nc.dram_tensor("Wr1", [hid_c, hid_c], dt.float32, kind="ExternalInput")
    b1_in = nc.dram_tensor("b1", [1, hid_c], dt.float32, kind="ExternalInput")
    wl2_in = nc.dram_tensor("Wl2", [hid_c, out_c], dt.float32, kind="ExternalInput")
    wr2_in = nc.dram_tensor("Wr2", [hid_c, out_c], dt.float32, kind="ExternalInput")
    b2_in = nc.dram_tensor("b2", [1, out_c], dt.float32, kind="ExternalInput")

    xfinal_out = nc.dram_tensor("xfinal", [npc, out_c], dt.float32, kind="ExternalOutput")
    out1_out = nc.dram_tensor("out1", [npc, hid_c], dt.float32, kind="ExternalOutput")
    g_out = nc.dram_tensor("g", [npc, hid_c], dt.float32, kind="ExternalOutput")

    with tile.TileContext(nc) as tc:
        with (
            tc.tile_pool(name="const", bufs=1) as cpool,
            tc.tile_pool(name="state", bufs=1) as spool,
            tc.tile_pool(name="work", bufs=3) as wpool,
            tc.tile_pool(name="msg", bufs=4) as mpool,
            tc.tile_pool(name="ps1", bufs=1, space="PSUM") as ps1,
            tc.tile_pool(name="ps2", bufs=2, space="PSUM") as ps2,
            tc.tile_pool(name="dram", bufs=1, space="DRAM") as dpool,
        ):
            nc.gpsimd.load_library(library_config.mlp)

            def ld(name, ap_src, shape, dtype, pool=cpool, tag=None):
                t = pool.tile(shape, dtype, tag=tag or name)
                nc.sync.dma_start(t[:], ap_src)
                return t

            ident = ld("identc", ident_in[:], [128, 128], dt.float32)
            iota_bf = ld("iotabfc", iota_bf_in[:], [128, nbmax * 128], dt.bfloat16)
            iota_f = ld("iotafc", iota_f_in[:], [128, nbmax * 128], dt.float32)
            ones_row = cpool.tile([1, 128], dt.float32, tag="onesrow")
            nc.vector.memset(ones_row[:], 1.0)
            ones_col = cpool.tile([128, 1], dt.bfloat16, tag="onescol")
            nc.vector.memset(ones_col[:], 1.0)

            wl0 = [ld(f"wl0_{j}", wl0_in[j * 128:(j + 1) * 128, :], [128, hid_c], dt.float32) for j in range(in_c // 128)]
            wr0 = [ld(f"wr0_{j}", wr0_in[j * 128:(j + 1) * 128, :], [128, hid_c], dt.float32) for j in range(in_c // 128)]
            wl1 = [ld("wl1", wl1_in[:], [hid_c, hid_c], dt.float32)]
            wr1 = [ld("wr1", wr1_in[:], [hid_c, hid_c], dt.float32)]
            wl2 = [ld("wl2", wl2_in[:], [hid_c, out_c], dt.float32)]
            wr2 = [ld("wr2", wr2_in[:], [hid_c, out_c], dt.float32)]
            b0t = ld("b0t", b0_in[:], [1, hid_c], dt.float32)
            b1t = ld("b1t", b1_in[:], [1, hid_c], dt.float32)
            b2t = ld("b2t", b2_in[:], [1, out_c], dt.float32)

            idxs = spool.tile([128, L // 16], dt.int16, tag="idxs")
            nc.sync.dma_start(idxs[:], idx_in[:])
            dloc = spool.tile([128, Lb], dt.bfloat16, tag="dloc")
            nc.sync.dma_start(dloc[:], dloc_in[:])
            invdeg = spool.tile([128, n_tiles], dt.float32, tag="invdeg")

            def phase_a(in_provider, wls, wrs, bt, cin, cout, tabown, projr, pl_dtype):
                ncin = cin // 128
                for t in range(n_tiles):
                    rows = P if t < n_tiles - 1 else rows_last
                    xt = in_provider(t, rows)
                    xTs = []
                    for j in range(ncin):
                        psT = ps2.tile([128, 128], dt.float32, tag="ptpose", space="PSUM")
                        nc.tensor.transpose(psT[:], xt[:, j * 128:(j + 1) * 128], ident[:])
                        xT = wpool.tile([128, 128], dt.float32, tag="xT")
                        nc.vector.tensor_copy(xT[:], psT[:])
                        xTs.append(xT)
                    psl = ps1.tile([128, cout], dt.float32, tag="ppl", space="PSUM")
                    psr = ps1.tile([128, cout], dt.float32, tag="ppr", space="PSUM")
                    for j in range(ncin):
                        nc.tensor.matmul(out=psl[:], lhsT=xTs[j][:], rhs=wls[j][:],
                                         start=(j == 0), stop=(j == ncin - 1))
                    for j in range(ncin):
                        nc.tensor.matmul(out=psr[:], lhsT=xTs[j][:], rhs=wrs[j][:],
                                         start=(j == 0), stop=False)
                    nc.tensor.matmul(out=psr[:], lhsT=ones_row[:], rhs=bt[:],
                                     start=False, stop=True)
                    pl = wpool.tile([128, cout], pl_dtype, tag="pl")
                    nc.vector.tensor_copy(pl[:], psl[:])
                    nc.sync.dma_start(tabown[t * P:t * P + rows, :], pl[:rows, :])
                    nc.vector.tensor_copy(projr[:, t, :cout], psr[:])

            def phase_c(layer, tab_full, cout, use_bf16, projr, out_writer):
                for t in range(n_tiles):
                    rows = P if t < n_tiles - 1 else rows_last
                    groups = []
                    for c in range(n_chunks):
                        gi = t * n_chunks + c
                        if S[gi] > 0:
                            groups.append((c, int(S[gi]), int(off[gi])))
                    acc = ps2.tile([128, cout], dt.float32, tag="acc", space="PSUM")
                    if layer == 0:
                        degp = ps1.tile([128, 1], dt.float32, tag="deg", space="PSUM")
                    total_blocks = sum(sg // P for _, sg, _ in groups)
                    bi = 0
                    for (c, sg, offg) in groups:
                        nb = sg // P
                        mdt = dt.bfloat16 if use_bf16 else dt.float32
                        msg = mpool.tile([128, nb, cout], mdt, tag="msg")
                        nc.gpsimd.dma_gather(
                            msg[:],
                            tab_full[c * chunk:(c + 1) * chunk, :],
                            idxs[:, offg // 16: offg // 16 + sg // 16],
                            sg, sg, cout,
                        )
                        B0 = offg // P
                        dl_b = dloc[:, B0:B0 + nb].unsqueeze(2).to_broadcast([128, nb, 128])
                        if use_bf16:
                            sel = mpool.tile([128, nb, 128], dt.bfloat16, tag="sel")
                            io_b = iota_bf[:, :nb * 128].rearrange("p (b i) -> p b i", b=nb)
                            nc.vector.tensor_tensor(out=sel[:], in0=dl_b, in1=io_b,
                                                    op=mybir.AluOpType.is_equal)
                        else:
                            dlf = wpool.tile([128, nbmax], dt.float32, tag="dlf")
                            nc.vector.tensor_copy(dlf[:, :nb], dloc[:, B0:B0 + nb])
                            sel = mpool.tile([128, nb, 128], dt.float32, tag="self")
                            io_f = iota_f[:, :nb * 128].rearrange("p (b i) -> p b i", b=nb)
                            nc.vector.tensor_tensor(
                                out=sel[:],
                                in0=dlf[:, :nb].unsqueeze(2).to_broadcast([128, nb, 128]),
                                in1=io_f, op=mybir.AluOpType.is_equal)
                        for b in range(nb):
                            last = bi == total_blocks - 1
                            nc.tensor.matmul(out=acc[:], lhsT=sel[:, b, :], rhs=msg[:, b, :],
                                             start=(bi == 0), stop=last)
                            if layer == 0:
                                nc.tensor.matmul(out=degp[:], lhsT=sel[:, b, :], rhs=ones_col[:],
                                                 start=(bi == 0), stop=last)
                            bi += 1
                    if layer == 0:
                        dg = wpool.tile([128, 1], dt.float32, tag="dg")
                        nc.vector.tensor_scalar(out=dg[:], in0=degp[:], scalar1=1.0,
                                                scalar2=None, op0=mybir.AluOpType.max)
                        nc.vector.reciprocal(invdeg[:, t:t + 1], dg[:])
                    o1 = wpool.tile([128, cout], dt.float32, tag="o1")
                    nc.vector.tensor_tensor(out=o1[:], in0=acc[:],
                                            in1=invdeg[:, t:t + 1].to_broadcast([128, cout]),
                                            op=mybir.AluOpType.mult)
                    o2 = wpool.tile([128, cout], dt.float32, tag="o2")
                    nc.vector.tensor_tensor(out=o2[:], in0=o1[:], in1=projr[:, t, :cout],
                                            op=mybir.AluOpType.add)
                    out_writer(t, rows, o2)

            rg = [list(range(n_cores))]

            # ---------------- layer 0 ----------------
            tab0own = dpool.tile([npc, hid_c], dt.bfloat16, tag="t0o")
            tab0 = dpool.tile([n_nodes, hid_c], dt.bfloat16, tag="t0", addr_space="Shared")
            projr0 = spool.tile([128, n_tiles, hid_c], dt.float32, tag="projr")
            state_h = spool.tile([128, n_tiles, hid_c], dt.float32, tag="state")

            def x_provider(t, rows):
                xt = wpool.tile([128, in_c], dt.float32, tag="xin")
                if rows < P:
                    nc.vector.memset(xt[:], 0.0)
                nc.sync.dma_start(xt[:rows, :], x_own[t * P:t * P + rows, :])
                return xt

            phase_a(x_provider, wl0, wr0, b0t, in_c, hid_c, tab0own, projr0, dt.bfloat16)
            nc.gpsimd.collective_compute(
                "AllGather", mybir.AluOpType.bypass, replica_groups=rg,
                ins=[tab0own.opt()], outs=[tab0.opt()])

            def writer0(t, rows, o2):
                nc.vector.tensor_scalar(out=state_h[:, t, :], in0=o2[:], scalar1=0.0,
                                        scalar2=None, op0=mybir.AluOpType.max)

            phase_c(0, tab0, hid_c, True, projr0, writer0)

            # ---------------- layer 1 ----------------
            tab1own = dpool.tile([npc, hid_c], dt.bfloat16, tag="t1o")
            tab1 = dpool.tile([n_nodes, hid_c], dt.bfloat16, tag="t1", addr_space="Shared")
            projr1 = spool.tile([128, n_tiles, hid_c], dt.float32, tag="projr")
            state_g = spool.tile([128, n_tiles, hid_c], dt.float32, tag="state")

            phase_a(lambda t, rows: state_h[:, t, :], wl1, wr1, b1t, hid_c, hid_c,
                    tab1own, projr1, dt.bfloat16)
            nc.gpsimd.collective_compute(
                "AllGather", mybir.AluOpType.bypass, replica_groups=rg,
                ins=[tab1own.opt()], outs=[tab1.opt()])

            def writer1(t, rows, o2):
                nc.sync.dma_start(out1_out[t * P:t * P + rows, :], o2[:rows, :])
                nc.vector.tensor_scalar(out=state_g[:, t, :], in0=o2[:], scalar1=0.0,
                                        scalar2=None, op0=mybir.AluOpType.max)
                nc.sync.dma_start(g_out[t * P:t * P + rows, :], state_g[:rows, t, :])

            phase_c(1, tab1, hid_c, True, projr1, writer1)

            # ---------------- layer 2 ----------------
            tab2own = dpool.tile([npc, out_c], dt.float32, tag="t2o")
            tab2 = dpool.tile([n_nodes, out_c], dt.float32, tag="t2", addr_space="Shared")
            projr2 = spool.tile([128, n_tiles, hid_c], dt.float32, tag="projr")

            phase_a(lambda t, rows: state_g[:, t, :], wl2, wr2, b2t, hid_c, out_c,
                    tab2own, projr2, dt.float32)
            nc.gpsimd.collective_compute(
                "AllGather", mybir.AluOpType.bypass, replica_groups=rg,
                ins=[tab2own.opt()], outs=[tab2.opt()])

            def writer2(t, rows, o2):
                nc.sync.dma_start(xfinal_out[t * P:t * P + rows, :], o2[:rows, :])

            phase_c(2, tab2, out_c, False, projr2, writer2)

    nc.compile()
    return nc


def _run(inputs, cfg, trace=False):
    n_cores = cfg["n_cores"]
    n_nodes = cfg["n_nodes"]
    npc = n_nodes // n_cores

    edge_index = np.asarray(inputs["edge_index"])
    x = np.asarray(inputs["x"], dtype=np.float32)

    S, off, L, idx_list, dloc_list = _preprocess(edge_index, cfg)
    nbmax = int((S // P).max())

    iota_row = np.tile(np.arange(128, dtype=np.float32), nbmax)
    iota_bf = np.broadcast_to(iota_row, (128, nbmax * 128)).astype(ml_dtypes.bfloat16)
    iota_f = np.broadcast_to(iota_row, (128, nbmax * 128)).astype(np.float32)
    ident = np.eye(128, dtype=np.float32)

    w = {k: np.asarray(inputs[k], dtype=np.float32) for k in
         ("Wl0", "Wr0", "b0", "Wl1", "Wr1", "b1", "Wl2", "Wr2", "b2")}

    in_maps = []
    for k in range(n_cores):
        in_maps.append({
            "x_own": np.ascontiguousarray(x[k * npc:(k + 1) * npc]),
            "idx": idx_list[k],
            "dloc": dloc_list[k],
            "iotabf": iota_bf,
            "iotaf": iota_f,
            "ident": ident,
            "Wl0": w["Wl0"], "Wr0": w["Wr0"], "b0": w["b0"].reshape(1, -1),
            "Wl1": w["Wl1"], "Wr1": w["Wr1"], "b1": w["b1"].reshape(1, -1),
            "Wl2": w["Wl2"], "Wr2": w["Wr2"], "b2": w["b2"].reshape(1, -1),
        })

    nc = _build(S, off, L, cfg)
    res = run_bass_kernel_spmd(nc, in_maps, list(range(n_cores)), trace=trace)

    x_final = np.concatenate([res.results[k]["xfinal"] for k in range(n_cores)], axis=0)
    out1 = np.concatenate([res.results[k]["out1"] for k in range(n_cores)], axis=0)
    g = np.concatenate([res.results[k]["g"] for k in range(n_cores)], axis=0)
    return (x_final, out1, g), res


def kernel(**inputs):
    (x_final, out1, g), _ = _run(inputs, CFG, trace=False)
    return (x_final, out1, g)


# revision 3
# speedup vs baseline: 1.7874x; 1.7874x over previous
"""3-layer GraphSAGE (PyG SAGEConv, mean aggregation) on 8 Trainium2 NeuronCores.

Strategy (edge-cut graph partition, per the sharding hint):
  - Destination nodes sharded contiguously across 8 cores (12500 rows each).
  - Mean aggregation is linear, so each layer first projects its own node
    rows through Wl (x @ Wl, bf16), all-gathers the projected table (the
    halo exchange), then gathers per-edge source rows with dma_gather and
    segment-sums them on the tensor engine using one-hot selection-matrix
    matmuls accumulated in PSUM.
  - Degrees are computed on-device by an extra matmul against a ones vector
    during layer 0 (reused for all layers).
  - Edges are grouped by (dst 128-tile, source 25000-row chunk) so gather
    indices fit int16; groups are padded to a cross-core-uniform size
    (16-granular) so a single SPMD program serves all 8 cores; the last
    partial 128-block of each group is handled with a K-trimmed matmul.
  - Gathers rotate across 4 SWDGE queues to keep descriptors in flight.

kernel(**inputs) takes the full unsharded inputs and returns the full
(x_final, out, g) tuple, matching the reference.
"""

import numpy as np
import ml_dtypes

import concourse.bacc as bacc
import concourse.tile as tile
import concourse.mybir as mybir
from concourse import library_config
from concourse.bass_utils import run_bass_kernel_spmd

P = 128
N_QUEUES = 4

CFG = dict(
    n_nodes=100000,
    in_c=256,
    hid_c=128,
    out_c=64,
    n_cores=8,
    chunk=25000,
)


def _cdiv(a, b):
    return -(-a // b)


def _preprocess(edge_index, cfg):
    """Host-side integer graph preprocessing: partition edges by dst owner,
    group by (dst tile, src chunk), sort by src, pad to cross-core-uniform
    16-granular sizes. Returns per-core int16 gather indices (wrapped
    layout) and bf16 local-dst arrays plus the shared group size tables."""
    n_nodes = cfg["n_nodes"]
    n_cores = cfg["n_cores"]
    chunk = cfg["chunk"]
    npc = n_nodes // n_cores
    n_tiles = _cdiv(npc, P)
    n_chunks = _cdiv(n_nodes, chunk)
    n_groups = n_tiles * n_chunks

    src = edge_index[0].astype(np.int64)
    dst = edge_index[1].astype(np.int64)
    owner = dst // npc

    per_core = []
    G = np.zeros((n_cores, n_groups), np.int64)
    for k in range(n_cores):
        m = owner == k
        s = src[m]
        d = dst[m] - k * npc
        t = d // P
        c = s // chunk
        g = t * n_chunks + c
        o = np.lexsort((s, g))
        s, d, g = s[o], d[o], g[o]
        cnt = np.bincount(g, minlength=n_groups)
        G[k] = cnt
        per_core.append((s, d, g, cnt))

    # padded group sizes, uniform across cores, 16-granular
    S = (_cdiv(G.max(axis=0), 16) * 16).astype(np.int64)
    off16 = np.zeros_like(S)
    off16[1:] = np.cumsum(S)[:-1]
    L16 = int(S.sum())
    NB = _cdiv(S, P)  # blocks per group
    offB = np.zeros_like(NB)
    offB[1:] = np.cumsum(NB)[:-1]
    Lb = int(NB.sum())

    idx_list, dloc_list = [], []
    for k in range(n_cores):
        s, d, g, cnt = per_core[k]
        gstart = np.zeros(n_groups, np.int64)
        gstart[1:] = np.cumsum(cnt)[:-1]
        e_in_g = np.arange(len(s)) - gstart[g]
        pos16 = off16[g] + e_in_g
        lsrc = (s - (g % n_chunks) * chunk).astype(np.int16)
        idxw = np.zeros((16, L16 // 16), np.int16)
        idxw[pos16 % 16, pos16 // 16] = lsrc
        idx_list.append(np.tile(idxw, (8, 1)))
        # dloc blocks are 128-aligned per group (offB), independent of off16
        posB = offB[g] * P + e_in_g
        dl = np.full((P, Lb), 255.0, np.float32)
        dl[posB % P, posB // P] = (d % P).astype(np.float32)
        dloc_list.append(dl)

    return S, off16, L16, NB, offB, Lb, idx_list, dloc_list


def _build(S, off16, L16, NB, offB, Lb, cfg):
    """Build the SPMD Bass program (identical for all 8 cores)."""
    dt = mybir.dt
    n_nodes = cfg["n_nodes"]
    n_cores = cfg["n_cores"]
    chunk = cfg["chunk"]
    in_c = cfg["in_c"]
    hid_c = cfg["hid_c"]
    out_c = cfg["out_c"]
    npc = n_nodes // n_cores
    n_tiles = _cdiv(npc, P)
    rows_last = npc - (n_tiles - 1) * P
    n_chunks = _cdiv(n_nodes, chunk)

    AF = mybir.ActivationFunctionType

    nc = bacc.Bacc("TRN2", target_bir_lowering=False, num_swdge_queues=N_QUEUES)

    x_own = nc.dram_tensor("x_own", [npc, in_c], dt.float32, kind="ExternalInput")
    idx_in = nc.dram_tensor("idx", [128, L16 // 16], dt.int16, kind="ExternalInput")
    dloc_in = nc.dram_tensor("dloc", [128, Lb], dt.float32, kind="ExternalInput")
    iota_bf_in = nc.dram_tensor("iotabf", [128, 128], dt.bfloat16, kind="ExternalInput")
    ident_in = nc.dram_tensor("ident", [128, 128], dt.float32, kind="ExternalInput")
    wl0_in = nc.dram_tensor("Wl0", [in_c, hid_c], dt.float32, kind="ExternalInput")
    wr0_in = nc.dram_tensor("Wr0", [in_c, hid_c], dt.float32, kind="ExternalInput")
    b0_in = nc.dram_tensor("b0", [1, hid_c], dt.float32, kind="ExternalInput")
    wl1_in = nc.dram_tensor("Wl1", [hid_c, hid_c], dt.float32, kind="ExternalInput")
    wr1_in = nc.dram_tensor("Wr1", [hid_c, hid_c], dt.float32, kind="ExternalInput")
    b1_in = nc.dram_tensor("b1", [1, hid_c], dt.float32, kind="ExternalInput")
    wl2_in = nc.dram_tensor("Wl2", [hid_c, out_c], dt.float32, kind="ExternalInput")
    wr2_in = nc.dram_tensor("Wr2", [hid_c, out_c], dt.float32, kind="ExternalInput")
    b2_in = nc.dram_tensor("b2", [1, out_c], dt.float32, kind="ExternalInput")

    xfinal_out = nc.dram_tensor("xfinal", [npc, out_c], dt.float32, kind="ExternalOutput")
    out1_out = nc.dram_tensor("out1", [npc, hid_c], dt.float32, kind="ExternalOutput")
    g_out = nc.dram_tensor("g", [npc, hid_c], dt.float32, kind="ExternalOutput")

    qctr = [0]

    with tile.TileContext(nc) as tc:
        with (
            tc.tile_pool(name="const", bufs=1) as cpool,
            tc.tile_pool(name="state", bufs=1) as spool,
            tc.tile_pool(name="work", bufs=3) as wpool,
            tc.tile_pool(name="msg", bufs=6) as mpool,
            tc.tile_pool(name="sel", bufs=6) as selpool,
            tc.tile_pool(name="ps1", bufs=1, space="PSUM") as ps1,
            tc.tile_pool(name="ps2", bufs=2, space="PSUM") as ps2,
            tc.tile_pool(name="dram", bufs=1, space="DRAM") as dpool,
        ):
            nc.gpsimd.load_library(library_config.mlp)

            def ld(name, ap_src, shape, dtype, pool=cpool):
                t = pool.tile(shape, dtype, tag=name)
                nc.sync.dma_start(t[:], ap_src)
                return t

            ident = ld("identc", ident_in[:], [128, 128], dt.float32)
            iota_bf = ld("iotabfc", iota_bf_in[:], [128, 128], dt.bfloat16)
            ones_row = cpool.tile([1, 128], dt.float32, tag="onesrow")
            nc.vector.memset(ones_row[:], 1.0)
            ones_col = cpool.tile([128, 1], dt.bfloat16, tag="onescol")
            nc.vector.memset(ones_col[:], 1.0)

            wl0 = [ld(f"wl0_{j}", wl0_in[j * 128:(j + 1) * 128, :], [128, hid_c], dt.float32) for j in range(in_c // 128)]
            wr0 = [ld(f"wr0_{j}", wr0_in[j * 128:(j + 1) * 128, :], [128, hid_c], dt.float32) for j in range(in_c // 128)]
            wl1 = [ld("wl1", wl1_in[:], [hid_c, hid_c], dt.float32)]
            wr1 = [ld("wr1", wr1_in[:], [hid_c, hid_c], dt.float32)]
            wl2 = [ld("wl2", wl2_in[:], [hid_c, out_c], dt.float32)]
            wr2 = [ld("wr2", wr2_in[:], [hid_c, out_c], dt.float32)]
            b0t = ld("b0t", b0_in[:], [1, hid_c], dt.float32)
            b1t = ld("b1t", b1_in[:], [1, hid_c], dt.float32)
            b2t = ld("b2t", b2_in[:], [1, out_c], dt.float32)

            idxs = spool.tile([128, L16 // 16], dt.int16, tag="idxs")
            nc.sync.dma_start(idxs[:], idx_in[:])
            dloc = spool.tile([128, Lb], dt.float32, tag="dloc")
            nc.sync.dma_start(dloc[:], dloc_in[:])
            invdeg = spool.tile([128, n_tiles], dt.float32, tag="invdeg")

            def phase_a(in_provider, wls, wrs, bt, cin, cout, tabown, tabown_c, projr):
                ncin = cin // 128
                for t in range(n_tiles):
                    rows = P if t < n_tiles - 1 else rows_last
                    xt = in_provider(t, rows)
                    xTs = []
                    for j in range(ncin):
                        psT = ps2.tile([128, 128], dt.float32, tag="ptpose", space="PSUM")
                        nc.tensor.transpose(psT[:], xt[:, j * 128:(j + 1) * 128], ident[:])
                        xT = wpool.tile([128, 128], dt.float32, tag="xT")
                        nc.scalar.activation(xT[:], psT[:], AF.Copy)
                        xTs.append(xT)
                    psl = ps1.tile([128, cout], dt.float32, tag="ppl", space="PSUM")
                    psr = ps1.tile([128, cout], dt.float32, tag="ppr", space="PSUM")
                    for j in range(ncin):
                        nc.tensor.matmul(out=psl[:], lhsT=xTs[j][:], rhs=wls[j][:],
                                         start=(j == 0), stop=(j == ncin - 1))
                    for j in range(ncin):
                        nc.tensor.matmul(out=psr[:], lhsT=xTs[j][:], rhs=wrs[j][:],
                                         start=(j == 0), stop=False)
                    nc.tensor.matmul(out=psr[:], lhsT=ones_row[:], rhs=bt[:],
                                     start=False, stop=True)
                    pl = wpool.tile([128, cout], dt.bfloat16, tag="pl")
                    nc.scalar.activation(pl[:], psl[:], AF.Copy)
                    nc.sync.dma_start(tabown[t * P:t * P + rows, :tabown_c], pl[:rows, :])
                    nc.scalar.activation(projr[:, t, :cout], psr[:], AF.Copy)

            def phase_c(layer, tab_full, cout, projr, out_writer):
                for t in range(n_tiles):
                    rows = P if t < n_tiles - 1 else rows_last
                    groups = []
                    for c in range(n_chunks):
                        gi = t * n_chunks + c
                        if S[gi] > 0:
                            groups.append((c, int(S[gi]), int(off16[gi]), int(offB[gi])))
                    acc = ps2.tile([128, cout], dt.float32, tag="acc", space="PSUM")
                    if layer == 0:
                        degp = ps1.tile([128, 1], dt.float32, tag="deg", space="PSUM")
                    total_blocks = sum(_cdiv(sg, P) for _, sg, _, _ in groups)
                    bi = 0
                    for (c, sg, o16, oB) in groups:
                        nb = _cdiv(sg, P)
                        msg = mpool.tile([128, nb, hid_c], dt.bfloat16, tag="msg")
                        nc.gpsimd.dma_gather(
                            msg[:],
                            tab_full[c * chunk:(c + 1) * chunk, :],
                            idxs[:, o16 // 16: (o16 + sg) // 16],
                            sg, sg, hid_c,
                            queue_num=qctr[0] % N_QUEUES,
                        )
                        qctr[0] += 1
                        for b in range(nb):
                            k = P if b < nb - 1 else sg - (nb - 1) * P
                            sel = selpool.tile([128, 128], dt.bfloat16, tag="sel")
                            nc.vector.tensor_scalar(
                                out=sel[:], in0=iota_bf[:],
                                scalar1=dloc[:, oB + b: oB + b + 1], scalar2=None,
                                op0=mybir.AluOpType.is_equal)
                            last = bi == total_blocks - 1
                            nc.tensor.matmul(out=acc[:], lhsT=sel[:k, :], rhs=msg[:k, b, :cout],
                                             start=(bi == 0), stop=last)
                            if layer == 0:
                                nc.tensor.matmul(out=degp[:], lhsT=sel[:k, :], rhs=ones_col[:k, :],
                                                 start=(bi == 0), stop=last)
                            bi += 1
                    if layer == 0:
                        dg = wpool.tile([128, 1], dt.float32, tag="dg")
                        nc.vector.tensor_scalar(out=dg[:], in0=degp[:], scalar1=1.0,
                                                scalar2=None, op0=mybir.AluOpType.max)
                        nc.vector.reciprocal(invdeg[:, t:t + 1], dg[:])
                    o1 = wpool.tile([128, cout], dt.float32, tag="o1")
                    nc.scalar.activation(o1[:], acc[:], AF.Copy, scale=invdeg[:, t:t + 1])
                    o2 = wpool.tile([128, cout], dt.float32, tag="o2")
                    nc.vector.tensor_tensor(out=o2[:], in0=o1[:], in1=projr[:, t, :cout],
                                            op=mybir.AluOpType.add)
                    out_writer(t, rows, o2)

            rg = [list(range(n_cores))]

            # ---------------- layer 0 ----------------
            tab0own = dpool.tile([npc, hid_c], dt.bfloat16, tag="t0o")
            tab0 = dpool.tile([n_nodes, hid_c], dt.bfloat16, tag="t0", addr_space="Shared")
            projr0 = spool.tile([128, n_tiles, hid_c], dt.float32, tag="projr")
            state_h = spool.tile([128, n_tiles, hid_c], dt.float32, tag="state")

            def x_provider(t, rows):
                xt = wpool.tile([128, in_c], dt.float32, tag="xin")
                if rows < P:
                    nc.vector.memset(xt[:], 0.0)
                nc.sync.dma_start(xt[:rows, :], x_own[t * P:t * P + rows, :])
                return xt

            phase_a(x_provider, wl0, wr0, b0t, in_c, hid_c, tab0own, hid_c, projr0)
            nc.gpsimd.collective_compute(
                "AllGather", mybir.AluOpType.bypass, replica_groups=rg,
                ins=[tab0own.opt()], outs=[tab0.opt()])

            def writer0(t, rows, o2):
                nc.scalar.activation(state_h[:, t, :], o2[:], AF.Relu)

            phase_c(0, tab0, hid_c, projr0, writer0)

            # ---------------- layer 1 ----------------
            tab1own = dpool.tile([npc, hid_c], dt.bfloat16, tag="t1o")
            tab1 = dpool.tile([n_nodes, hid_c], dt.bfloat16, tag="t1", addr_space="Shared")
            projr1 = spool.tile([128, n_tiles, hid_c], dt.float32, tag="projr")
            state_g = spool.tile([128, n_tiles, hid_c], dt.float32, tag="state")

            phase_a(lambda t, rows: state_h[:, t, :], wl1, wr1, b1t, hid_c, hid_c,
                    tab1own, hid_c, projr1)
            nc.gpsimd.collective_compute(
                "AllGather", mybir.AluOpType.bypass, replica_groups=rg,
                ins=[tab1own.opt()], outs=[tab1.opt()])

            def writer1(t, rows, o2):
                nc.sync.dma_start(out1_out[t * P:t * P + rows, :], o2[:rows, :])
                nc.scalar.activation(state_g[:, t, :], o2[:], AF.Relu)
                nc.sync.dma_start(g_out[t * P:t * P + rows, :], state_g[:rows, t, :])

            phase_c(1, tab1, hid_c, projr1, writer1)

            # ---------------- layer 2 ----------------
            # table rows are hid_c wide; only the first out_c columns are real
            # (the rest is never read by the matmuls), keeping gather rows at
            # 256B and everything bf16.
            tab2own = dpool.tile([npc, hid_c], dt.bfloat16, tag="t2o")
            tab2 = dpool.tile([n_nodes, hid_c], dt.bfloat16, tag="t2", addr_space="Shared")
            projr2 = spool.tile([128, n_tiles, hid_c], dt.float32, tag="projr")

            phase_a(lambda t, rows: state_g[:, t, :], wl2, wr2, b2t, hid_c, out_c,
                    tab2own, out_c, projr2)
            nc.gpsimd.collective_compute(
                "AllGather", mybir.AluOpType.bypass, replica_groups=rg,
                ins=[tab2own.opt()], outs=[tab2.opt()])

            def writer2(t, rows, o2):
                nc.sync.dma_start(xfinal_out[t * P:t * P + rows, :], o2[:rows, :])

            phase_c(2, tab2, out_c, projr2, writer2)

    nc.compile()
    return nc


def _run(inputs, cfg, trace=False):
    n_cores = cfg["n_cores"]
    n_nodes = cfg["n_nodes"]
    npc = n_nodes // n_cores

    edge_index = np.asarray(inputs["edge_index"])
    x = np.asarray(inputs["x"], dtype=np.float32)

    S, off16, L16, NB, offB, Lb, idx_list, dloc_list = _preprocess(edge_index, cfg)

    iota_row = np.arange(128, dtype=np.float32)
    iota_bf = np.broadcast_to(iota_row, (128, 128)).astype(ml_dtypes.bfloat16)
    ident = np.eye(128, dtype=np.float32)

    w = {k: np.asarray(inputs[k], dtype=np.float32) for k in
         ("Wl0", "Wr0", "b0", "Wl1", "Wr1", "b1", "Wl2", "Wr2", "b2")}

    in_maps = []
    for k in range(n_cores):
        in_maps.append({
            "x_own": np.ascontiguousarray(x[k * npc:(k + 1) * npc]),
            "idx": idx_list[k],
            "dloc": dloc_list[k],
            "iotabf": iota_bf,
            "ident": ident,
            "Wl0": w["Wl0"], "Wr0": w["Wr0"], "b0": w["b0"].reshape(1, -1),
            "Wl1": w["Wl1"], "Wr1": w["Wr1"], "b1": w["b1"].reshape(1, -1),
            "Wl2": w["Wl2"], "Wr2": w["Wr2"], "b2": w["b2"].reshape(1, -1),
        })

    nc = _build(S, off16, L16, NB, offB, Lb, cfg)
    res = run_bass_kernel_spmd(nc, in_maps, list(range(n_cores)), trace=trace)

    x_final = np.concatenate([res.results[k]["xfinal"] for k in range(n_cores)], axis=0)
    out1 = np.concatenate([res.results[k]["out1"] for k in range(n_cores)], axis=0)
    g = np.concatenate([res.results[k]["g"] for k in range(n_cores)], axis=0)
    return (x_final, out1, g), res


def kernel(**inputs):
    (x_final, out1, g), _ = _run(inputs, CFG, trace=False)
    return (x_final, out1, g)


# revision 4
# speedup vs baseline: 2.6157x; 1.4634x over previous
"""3-layer GraphSAGE (PyG SAGEConv, mean aggregation) on 8 Trainium2 NeuronCores.

Strategy (edge-cut graph partition, per the sharding hint):
  - Destination nodes sharded contiguously across 8 cores (12500 rows each).
  - Mean aggregation is linear, so each layer first projects its own node
    rows through Wl (x @ Wl, bf16), all-gathers the projected table (the
    halo exchange), then gathers per-edge source rows with dma_gather and
    segment-sums them on the tensor engine using one-hot selection-matrix
    matmuls accumulated in PSUM.
  - Degrees are computed on-device by an extra matmul against a ones vector
    during layer 0 (reused for all layers).
  - Edges are grouped by (dst 128-tile, source 25000-row chunk) so gather
    indices fit int16; groups are padded to a cross-core-uniform size
    (16-granular) so a single SPMD program serves all 8 cores; the last
    partial 128-block of each group is handled with a K-trimmed matmul.
  - Gathers rotate across 4 SWDGE queues to keep descriptors in flight.

kernel(**inputs) takes the full unsharded inputs and returns the full
(x_final, out, g) tuple, matching the reference.
"""

import numpy as np
import ml_dtypes

import concourse.bacc as bacc
import concourse.tile as tile
import concourse.mybir as mybir
from concourse import library_config
from concourse.bass_utils import run_bass_kernel_spmd

P = 128
N_QUEUES = 4

CFG = dict(
    n_nodes=100000,
    in_c=256,
    hid_c=128,
    out_c=64,
    n_cores=8,
    chunk=25000,
)


def _cdiv(a, b):
    return -(-a // b)


def _preprocess(edge_index, cfg):
    """Host-side integer graph preprocessing: partition edges by dst owner,
    group by (dst tile, src chunk), sort by src, pad to cross-core-uniform
    16-granular sizes. Returns per-core int16 gather indices (wrapped
    layout) and bf16 local-dst arrays plus the shared group size tables."""
    n_nodes = cfg["n_nodes"]
    n_cores = cfg["n_cores"]
    chunk = cfg["chunk"]
    npc = n_nodes // n_cores
    n_tiles = _cdiv(npc, P)
    n_chunks = _cdiv(n_nodes, chunk)
    n_groups = n_tiles * n_chunks

    src = edge_index[0].astype(np.int64)
    dst = edge_index[1].astype(np.int64)
    owner = dst // npc

    per_core = []
    G = np.zeros((n_cores, n_groups), np.int64)
    for k in range(n_cores):
        m = owner == k
        s = src[m]
        d = dst[m] - k * npc
        t = d // P
        c = s // chunk
        g = t * n_chunks + c
        o = np.lexsort((s, g))
        s, d, g = s[o], d[o], g[o]
        cnt = np.bincount(g, minlength=n_groups)
        G[k] = cnt
        per_core.append((s, d, g, cnt))

    # padded group sizes, uniform across cores, 16-granular
    S = (_cdiv(G.max(axis=0), 16) * 16).astype(np.int64)
    off16 = np.zeros_like(S)
    off16[1:] = np.cumsum(S)[:-1]
    L16 = int(S.sum())
    NB = _cdiv(S, P)  # blocks per group
    offB = np.zeros_like(NB)
    offB[1:] = np.cumsum(NB)[:-1]
    Lb = int(NB.sum())

    idx_list, dloc_list = [], []
    for k in range(n_cores):
        s, d, g, cnt = per_core[k]
        gstart = np.zeros(n_groups, np.int64)
        gstart[1:] = np.cumsum(cnt)[:-1]
        e_in_g = np.arange(len(s)) - gstart[g]
        pos16 = off16[g] + e_in_g
        lsrc = (s - (g % n_chunks) * chunk).astype(np.int16)
        idxw = np.zeros((16, L16 // 16), np.int16)
        idxw[pos16 % 16, pos16 // 16] = lsrc
        idx_list.append(np.tile(idxw, (8, 1)))
        # dloc blocks are 128-aligned per group (offB), independent of off16
        posB = offB[g] * P + e_in_g
        dl = np.full((P, Lb), 255.0, np.float32)
        dl[posB % P, posB // P] = (d % P).astype(np.float32)
        dloc_list.append(dl.astype(ml_dtypes.bfloat16))

    return S, off16, L16, NB, offB, Lb, idx_list, dloc_list


def _build(S, off16, L16, NB, offB, Lb, cfg):
    """Build the SPMD Bass program (identical for all 8 cores)."""
    dt = mybir.dt
    n_nodes = cfg["n_nodes"]
    n_cores = cfg["n_cores"]
    chunk = cfg["chunk"]
    in_c = cfg["in_c"]
    hid_c = cfg["hid_c"]
    out_c = cfg["out_c"]
    npc = n_nodes // n_cores
    n_tiles = _cdiv(npc, P)
    rows_last = npc - (n_tiles - 1) * P
    n_chunks = _cdiv(n_nodes, chunk)

    AF = mybir.ActivationFunctionType

    nc = bacc.Bacc("TRN2", target_bir_lowering=False, num_swdge_queues=N_QUEUES)

    x_own = nc.dram_tensor("x_own", [npc, in_c], dt.float32, kind="ExternalInput")
    idx_in = nc.dram_tensor("idx", [128, L16 // 16], dt.int16, kind="ExternalInput")
    dloc_in = nc.dram_tensor("dloc", [128, Lb], dt.bfloat16, kind="ExternalInput")
    iota_bf_in = nc.dram_tensor("iotabf", [128, 128], dt.bfloat16, kind="ExternalInput")
    ident_in = nc.dram_tensor("ident", [128, 128], dt.float32, kind="ExternalInput")
    wl0_in = nc.dram_tensor("Wl0", [in_c, hid_c], dt.float32, kind="ExternalInput")
    wr0_in = nc.dram_tensor("Wr0", [in_c, hid_c], dt.float32, kind="ExternalInput")
    b0_in = nc.dram_tensor("b0", [1, hid_c], dt.float32, kind="ExternalInput")
    wl1_in = nc.dram_tensor("Wl1", [hid_c, hid_c], dt.float32, kind="ExternalInput")
    wr1_in = nc.dram_tensor("Wr1", [hid_c, hid_c], dt.float32, kind="ExternalInput")
    b1_in = nc.dram_tensor("b1", [1, hid_c], dt.float32, kind="ExternalInput")
    wl2_in = nc.dram_tensor("Wl2", [hid_c, out_c], dt.float32, kind="ExternalInput")
    wr2_in = nc.dram_tensor("Wr2", [hid_c, out_c], dt.float32, kind="ExternalInput")
    b2_in = nc.dram_tensor("b2", [1, out_c], dt.float32, kind="ExternalInput")

    xfinal_out = nc.dram_tensor("xfinal", [npc, out_c], dt.float32, kind="ExternalOutput")
    out1_out = nc.dram_tensor("out1", [npc, hid_c], dt.float32, kind="ExternalOutput")
    g_out = nc.dram_tensor("g", [npc, hid_c], dt.float32, kind="ExternalOutput")

    qctr = [0]

    with tile.TileContext(nc) as tc:
        with (
            tc.tile_pool(name="const", bufs=1) as cpool,
            tc.tile_pool(name="state", bufs=1) as spool,
            tc.tile_pool(name="work", bufs=3) as wpool,
            tc.tile_pool(name="msg", bufs=6) as mpool,
            tc.tile_pool(name="sel", bufs=6) as selpool,
            tc.tile_pool(name="ps1", bufs=1, space="PSUM") as ps1,
            tc.tile_pool(name="ps2", bufs=2, space="PSUM") as ps2,
            tc.tile_pool(name="dram", bufs=1, space="DRAM") as dpool,
        ):
            nc.gpsimd.load_library(library_config.mlp)

            def ld(name, ap_src, shape, dtype, pool=cpool):
                t = pool.tile(shape, dtype, tag=name)
                nc.sync.dma_start(t[:], ap_src)
                return t

            ident = ld("identc", ident_in[:], [128, 128], dt.float32)
            iota_bf = ld("iotabfc", iota_bf_in[:], [128, 128], dt.bfloat16)
            ones_row = cpool.tile([1, 128], dt.float32, tag="onesrow")
            nc.vector.memset(ones_row[:], 1.0)
            ones_col = cpool.tile([128, 1], dt.bfloat16, tag="onescol")
            nc.vector.memset(ones_col[:], 1.0)

            wl0 = [ld(f"wl0_{j}", wl0_in[j * 128:(j + 1) * 128, :], [128, hid_c], dt.float32) for j in range(in_c // 128)]
            wr0 = [ld(f"wr0_{j}", wr0_in[j * 128:(j + 1) * 128, :], [128, hid_c], dt.float32) for j in range(in_c // 128)]
            wl1 = [ld("wl1", wl1_in[:], [hid_c, hid_c], dt.float32)]
            wr1 = [ld("wr1", wr1_in[:], [hid_c, hid_c], dt.float32)]
            wl2 = [ld("wl2", wl2_in[:], [hid_c, out_c], dt.float32)]
            wr2 = [ld("wr2", wr2_in[:], [hid_c, out_c], dt.float32)]
            b0t = ld("b0t", b0_in[:], [1, hid_c], dt.float32)
            b1t = ld("b1t", b1_in[:], [1, hid_c], dt.float32)
            b2t = ld("b2t", b2_in[:], [1, out_c], dt.float32)

            idxs = spool.tile([128, L16 // 16], dt.int16, tag="idxs")
            nc.sync.dma_start(idxs[:], idx_in[:])
            dloc = spool.tile([128, Lb], dt.bfloat16, tag="dloc")
            nc.sync.dma_start(dloc[:], dloc_in[:])
            invdeg = spool.tile([128, n_tiles], dt.float32, tag="invdeg")

            def phase_a(in_provider, wls, wrs, bt, cin, cout, tabown, tabown_c, projr):
                ncin = cin // 128
                for t in range(n_tiles):
                    rows = P if t < n_tiles - 1 else rows_last
                    xt = in_provider(t, rows)
                    xTs = []
                    for j in range(ncin):
                        psT = ps2.tile([128, 128], dt.float32, tag="ptpose", space="PSUM")
                        nc.tensor.transpose(psT[:], xt[:, j * 128:(j + 1) * 128], ident[:])
                        xT = wpool.tile([128, 128], dt.float32, tag="xT")
                        nc.scalar.activation(xT[:], psT[:], AF.Copy)
                        xTs.append(xT)
                    psl = ps1.tile([128, cout], dt.float32, tag="ppl", space="PSUM")
                    psr = ps1.tile([128, cout], dt.float32, tag="ppr", space="PSUM")
                    for j in range(ncin):
                        nc.tensor.matmul(out=psl[:], lhsT=xTs[j][:], rhs=wls[j][:],
                                         start=(j == 0), stop=(j == ncin - 1))
                    for j in range(ncin):
                        nc.tensor.matmul(out=psr[:], lhsT=xTs[j][:], rhs=wrs[j][:],
                                         start=(j == 0), stop=False)
                    nc.tensor.matmul(out=psr[:], lhsT=ones_row[:], rhs=bt[:],
                                     start=False, stop=True)
                    pl = wpool.tile([128, cout], dt.bfloat16, tag="pl")
                    nc.scalar.activation(pl[:], psl[:], AF.Copy)
                    nc.sync.dma_start(tabown[t * P:t * P + rows, :tabown_c], pl[:rows, :])
                    nc.scalar.activation(projr[:, t, :cout], psr[:], AF.Copy)

            def phase_c(layer, tab_full, cout, projr, out_writer):
                for t in range(n_tiles):
                    rows = P if t < n_tiles - 1 else rows_last
                    groups = []
                    for c in range(n_chunks):
                        gi = t * n_chunks + c
                        if S[gi] > 0:
                            groups.append((c, int(S[gi]), int(off16[gi]), int(offB[gi])))
                    acc = ps2.tile([128, cout], dt.float32, tag="acc", space="PSUM")
                    if layer == 0:
                        degp = ps1.tile([128, 1], dt.float32, tag="deg", space="PSUM")
                    total_blocks = sum(_cdiv(sg, P) for _, sg, _, _ in groups)
                    bi = 0
                    for (c, sg, o16, oB) in groups:
                        nb = _cdiv(sg, P)
                        msg = mpool.tile([128, nb, hid_c], dt.bfloat16, tag="msg")
                        nc.gpsimd.dma_gather(
                            msg[:],
                            tab_full[c * chunk:(c + 1) * chunk, :],
                            idxs[:, o16 // 16: (o16 + sg) // 16],
                            sg, sg, hid_c,
                            queue_num=qctr[0] % N_QUEUES,
                        )
                        qctr[0] += 1
                        sel = selpool.tile([128, nb, 128], dt.bfloat16, tag="sel")
                        nc.vector.tensor_tensor(
                            out=sel[:],
                            in0=dloc[:, oB:oB + nb].unsqueeze(2).to_broadcast([128, nb, 128]),
                            in1=iota_bf[:, :].unsqueeze(1).to_broadcast([128, nb, 128]),
                            op=mybir.AluOpType.is_equal)
                        for b in range(nb):
                            k = P if b < nb - 1 else sg - (nb - 1) * P
                            last = bi == total_blocks - 1
                            nc.tensor.matmul(out=acc[:], lhsT=sel[:k, b, :], rhs=msg[:k, b, :cout],
                                             start=(bi == 0), stop=last)
                            if layer == 0:
                                nc.tensor.matmul(out=degp[:], lhsT=sel[:k, b, :], rhs=ones_col[:k, :],
                                                 start=(bi == 0), stop=last)
                            bi += 1
                    if layer == 0:
                        dg = wpool.tile([128, 1], dt.float32, tag="dg")
                        nc.vector.tensor_scalar(out=dg[:], in0=degp[:], scalar1=1.0,
                                                scalar2=None, op0=mybir.AluOpType.max)
                        nc.vector.reciprocal(invdeg[:, t:t + 1], dg[:])
                    o1 = wpool.tile([128, cout], dt.float32, tag="o1")
                    nc.scalar.activation(o1[:], acc[:], AF.Copy, scale=invdeg[:, t:t + 1])
                    o2 = wpool.tile([128, cout], dt.float32, tag="o2")
                    nc.vector.tensor_tensor(out=o2[:], in0=o1[:], in1=projr[:, t, :cout],
                                            op=mybir.AluOpType.add)
                    out_writer(t, rows, o2)

            rg = [list(range(n_cores))]

            # ---------------- layer 0 ----------------
            tab0own = dpool.tile([npc, hid_c], dt.bfloat16, tag="t0o")
            tab0 = dpool.tile([n_nodes, hid_c], dt.bfloat16, tag="t0", addr_space="Shared")
            projr0 = spool.tile([128, n_tiles, hid_c], dt.float32, tag="projr")
            state_h = spool.tile([128, n_tiles, hid_c], dt.float32, tag="state")

            def x_provider(t, rows):
                xt = wpool.tile([128, in_c], dt.float32, tag="xin")
                if rows < P:
                    nc.vector.memset(xt[:], 0.0)
                nc.sync.dma_start(xt[:rows, :], x_own[t * P:t * P + rows, :])
                return xt

            phase_a(x_provider, wl0, wr0, b0t, in_c, hid_c, tab0own, hid_c, projr0)
            nc.gpsimd.collective_compute(
                "AllGather", mybir.AluOpType.bypass, replica_groups=rg,
                ins=[tab0own.opt()], outs=[tab0.opt()])

            def writer0(t, rows, o2):
                nc.scalar.activation(state_h[:, t, :], o2[:], AF.Relu)

            phase_c(0, tab0, hid_c, projr0, writer0)

            # ---------------- layer 1 ----------------
            tab1own = dpool.tile([npc, hid_c], dt.bfloat16, tag="t1o")
            tab1 = dpool.tile([n_nodes, hid_c], dt.bfloat16, tag="t1", addr_space="Shared")
            projr1 = spool.tile([128, n_tiles, hid_c], dt.float32, tag="projr")
            state_g = spool.tile([128, n_tiles, hid_c], dt.float32, tag="state")

            phase_a(lambda t, rows: state_h[:, t, :], wl1, wr1, b1t, hid_c, hid_c,
                    tab1own, hid_c, projr1)
            nc.gpsimd.collective_compute(
                "AllGather", mybir.AluOpType.bypass, replica_groups=rg,
                ins=[tab1own.opt()], outs=[tab1.opt()])

            def writer1(t, rows, o2):
                nc.sync.dma_start(out1_out[t * P:t * P + rows, :], o2[:rows, :])
                nc.scalar.activation(state_g[:, t, :], o2[:], AF.Relu)
                nc.sync.dma_start(g_out[t * P:t * P + rows, :], state_g[:rows, t, :])

            phase_c(1, tab1, hid_c, projr1, writer1)

            # ---------------- layer 2 ----------------
            # table rows are hid_c wide; only the first out_c columns are real
            # (the rest is never read by the matmuls), keeping gather rows at
            # 256B and everything bf16.
            tab2own = dpool.tile([npc, hid_c], dt.bfloat16, tag="t2o")
            tab2 = dpool.tile([n_nodes, hid_c], dt.bfloat16, tag="t2", addr_space="Shared")
            projr2 = spool.tile([128, n_tiles, hid_c], dt.float32, tag="projr")

            phase_a(lambda t, rows: state_g[:, t, :], wl2, wr2, b2t, hid_c, out_c,
                    tab2own, out_c, projr2)
            nc.gpsimd.collective_compute(
                "AllGather", mybir.AluOpType.bypass, replica_groups=rg,
                ins=[tab2own.opt()], outs=[tab2.opt()])

            def writer2(t, rows, o2):
                nc.sync.dma_start(xfinal_out[t * P:t * P + rows, :], o2[:rows, :])

            phase_c(2, tab2, out_c, projr2, writer2)

    nc.compile()
    return nc


def _run(inputs, cfg, trace=False):
    n_cores = cfg["n_cores"]
    n_nodes = cfg["n_nodes"]
    npc = n_nodes // n_cores

    edge_index = np.asarray(inputs["edge_index"])
    x = np.asarray(inputs["x"], dtype=np.float32)

    S, off16, L16, NB, offB, Lb, idx_list, dloc_list = _preprocess(edge_index, cfg)

    iota_row = np.arange(128, dtype=np.float32)
    iota_bf = np.broadcast_to(iota_row, (128, 128)).astype(ml_dtypes.bfloat16)
    ident = np.eye(128, dtype=np.float32)

    w = {k: np.asarray(inputs[k], dtype=np.float32) for k in
         ("Wl0", "Wr0", "b0", "Wl1", "Wr1", "b1", "Wl2", "Wr2", "b2")}

    in_maps = []
    for k in range(n_cores):
        in_maps.append({
            "x_own": np.ascontiguousarray(x[k * npc:(k + 1) * npc]),
            "idx": idx_list[k],
            "dloc": dloc_list[k],
            "iotabf": iota_bf,
            "ident": ident,
            "Wl0": w["Wl0"], "Wr0": w["Wr0"], "b0": w["b0"].reshape(1, -1),
            "Wl1": w["Wl1"], "Wr1": w["Wr1"], "b1": w["b1"].reshape(1, -1),
            "Wl2": w["Wl2"], "Wr2": w["Wr2"], "b2": w["b2"].reshape(1, -1),
        })

    nc = _build(S, off16, L16, NB, offB, Lb, cfg)
    res = run_bass_kernel_spmd(nc, in_maps, list(range(n_cores)), trace=trace)

    x_final = np.concatenate([res.results[k]["xfinal"] for k in range(n_cores)], axis=0)
    out1 = np.concatenate([res.results[k]["out1"] for k in range(n_cores)], axis=0)
    g = np.concatenate([res.results[k]["g"] for k in range(n_cores)], axis=0)
    return (x_final, out1, g), res


def kernel(**inputs):
    (x_final, out1, g), _ = _run(inputs, CFG, trace=False)
    return (x_final, out1, g)
